# revision 4
# baseline (speedup 1.0000x reference)
"""Llama3 attention layer (T=2048, 32 q heads / 8 kv heads, D=128, hidden 4096)
on 8 Trainium2 NeuronCores, tensor-parallel over heads.

Per-core shard: 4 q heads + 1 kv head (w_qkv columns), 512 w_o rows.
Each core computes a full [T, 4096] o_proj partial in bf16; the host sums
the 8 partials in f32 (the all-reduce of the row-parallel w_o).

Device algorithm (per core), matmuls in bf16 with fp32 PSUM accumulation:
  1. qkv^T = w_shard^T @ hs^T          -> [768, T]  (c on partitions)
     cb order: k first, then q0..q3, then v (k's rope off critical path)
  2. RoPE on q^T/k^T rows via duplicated cos/sin tables (DVE)
  3. V = transpose(v^T) via PE transposes
  4. S^T[s,t] = k^T.T @ q^T per head; exp on ACT into a wide P tile;
     0/1 mask on diagonal blocks (DVE)
  5. out^T[d,t] += V[s].T @ P^T[s,t]; denominator via DVE accumulation of
     P blocks + one ones-matmul per (head, tile); reciprocal_approx_fast;
     normalize reads PSUM directly
  6. o_proj emitted as 32 "units" per tile, interleaved into the NEXT
     tile's QKV/attention phases to keep the PE saturated
"""
import math
from collections import deque

import numpy as np
import ml_dtypes

import bass_rust
import concourse.bass as bass
import concourse.mybir as mybir
import concourse.tile as tile
from concourse.bass_utils import run_bass_kernel_spmd
from concourse.masks import make_identity
from concourse.vector_clock import ScopedClock

BF16 = mybir.dt.bfloat16
F32 = mybir.dt.float32
bf16 = ml_dtypes.bfloat16

T = 2048
HID = 4096
D = 128
NQH = 4           # q heads per core
CB = 6            # qkv col blocks of 128 (k, 4 q heads, v)
HCH = HID // 128  # 32 hidden chunks
TJ = 512          # t tile width
NJ = T // TJ      # 4 t tiles
SB = T // 128     # 16 s blocks
SCALE = 1.0 / math.sqrt(D)

_MAX_CTRL_WAITS = 1


def _install_drain_fix():
    """walrus in this image allows only 1 sem wait on CTRL (nop/drain)
    instructions; spread the Tile tail-drain's global-clock waits across
    preceding sync-engine NOPs."""
    if getattr(tile.TileContext, "_drain_fix_installed", False):
        return

    def _patched(self, tick_clock, wait_clock):
        nc = self.nc
        nops = [nc.sync.nop(nofuse=True, hint=f"drainw{i}") for i in range(32)]
        drain_inst = nc.sync.drain()
        wait_clock.add_sem_waits(
            drain_inst.ins, ScopedClock({None: tick_clock.global_clock})
        )
        si = drain_inst.ins.sync_info
        waits = list(si.on_wait) if si and si.on_wait else []
        if len(waits) > _MAX_CTRL_WAITS:
            chunks = [
                waits[i:i + _MAX_CTRL_WAITS]
                for i in range(0, len(waits), _MAX_CTRL_WAITS)
            ]
            drain_inst.ins.sync_info = bass_rust.SyncInfo(
                on_wait=chunks[-1], on_update=list(si.on_update or [])
            )
            for nop, chunk in zip(nops, chunks[:-1]):
                nop.ins.sync_info = bass_rust.SyncInfo(on_wait=chunk, on_update=[])
        nc.all_engine_barrier()
        assert self.sems is not None
        popped = nc._tile_sem_poison_stack.pop()
        assert popped is self._sem_poison
        nc.clear_and_free_semaphores(list(self.sems.allocated().values()))
        nc.all_engine_barrier()

    tile.TileContext._drain_and_barrier = _patched
    tile.TileContext._drain_fix_installed = True


def _fix_bir_waits(bir_json: bytes, max_waits: int = 1) -> bytes:
    """walrus in this image accepts very few sem waits per instruction.
    Split any instruction carrying more than `max_waits` waits by inserting
    same-engine NoOps ahead of it that carry the excess waits."""
    import json

    bir = json.loads(bir_json)
    for fn in bir["functions"]:
        for blk in fn["blocks"]:
            out = []
            for inst in blk["instructions"]:
                si = inst.get("sync_info")
                waits = (si or {}).get("on_wait") or []
                if len(waits) > max_waits:
                    chunks = [
                        waits[i:i + max_waits]
                        for i in range(0, len(waits), max_waits)
                    ]
                    for k, ch in enumerate(chunks[:-1]):
                        out.append(
                            {
                                "debug": inst.get("debug", 0),
                                "engine": inst["engine"],
                                "ins": [],
                                "name": f"{inst['name']}-w{k}",
                                "opcode": "NoOp",
                                "outs": [],
                                "sync_info": {"on_update": [], "on_wait": ch},
                            }
                        )
                    si["on_wait"] = chunks[-1]
                out.append(inst)
            blk["instructions"] = out
    return json.dumps(bir).encode()


def build_nc() -> bass.Bass:
    _install_drain_fix()
    nc = bass.Bass()

    hsT_d = nc.dram_tensor("hsT", [128, HCH, T], BF16, kind="ExternalInput")
    w_d = nc.dram_tensor("wqkv", [128, CB, HCH, 128], BF16, kind="ExternalInput")
    wo_d = nc.dram_tensor("wo", [128, NQH, HID], BF16, kind="ExternalInput")
    cos_d = nc.dram_tensor("cos2", [128, T], BF16, kind="ExternalInput")
    sin_d = nc.dram_tensor("sin2", [128, T], BF16, kind="ExternalInput")
    mask_d = nc.dram_tensor("masks", [128, 4, TJ], BF16, kind="ExternalInput")
    out_d = nc.dram_tensor("out", [T, HID], BF16, kind="ExternalOutput")

    with tile.TileContext(nc) as tc:
        with (
            tc.tile_pool(name="const", bufs=1) as constp,
            tc.tile_pool(name="acts", bufs=1) as actp,
            tc.tile_pool(name="hst", bufs=1) as hstp,
            tc.tile_pool(name="qp", bufs=8) as qp,
            tc.tile_pool(name="otp", bufs=8) as otp,
            tc.tile_pool(name="Pp", bufs=1) as Pp,
            tc.tile_pool(name="qkt", bufs=2) as qktp,
            tc.tile_pool(name="rtmp", bufs=3) as rtp,
            tc.tile_pool(name="accp", bufs=4) as accp,
            tc.tile_pool(name="rcp", bufs=4) as rcpp,
            tc.tile_pool(name="outp", bufs=2) as outp,
            tc.tile_pool(name="pss", bufs=3, space="PSUM") as pssp,
            tc.tile_pool(name="pso", bufs=2, space="PSUM") as psop,
            tc.tile_pool(name="shp", bufs=3, space="PSUM") as shp,
        ):
            cos_sb = constp.tile([128, T], BF16, tag="cos")
            sin_sb = constp.tile([128, T], BF16, tag="sin")
            mask_sb = constp.tile([128, 4, TJ], BF16, tag="mask")
            ones_sb = constp.tile([128, 128], BF16, tag="ones")
            ident_sb = constp.tile([128, 128], BF16, tag="ident")

            # persistent activations
            w_sb = actp.tile([128, CB, HCH, 128], BF16, tag="w")
            wo_sb = actp.tile([128, NQH, HID], BF16, tag="wo")
            kT_sb = actp.tile([128, T], BF16, tag="kT")
            vT_sb = actp.tile([128, T], BF16, tag="vT")
            v_sb = [
                actp.tile([128, 128], BF16, tag=f"v{i}", name=f"v{i}")
                for i in range(SB)
            ]

            # ---- initial DMA schedule (j=0): interleave w / hst / consts so
            # the first matmuls start as early as possible.
            hst0 = hstp.tile([128, HCH, TJ], BF16, tag="hst", name="hst0")
            nc.sync.dma_start(w_sb[:, 0, 0:8, :], w_d[:, 0, 0:8, :])
            nc.sync.dma_start(hst0[:, 0:8, :], hsT_d[:, 0:8, 0:TJ])
            nc.sync.dma_start(w_sb[:, 0, 8:32, :], w_d[:, 0, 8:32, :])
            nc.sync.dma_start(hst0[:, 8:16, :], hsT_d[:, 8:16, 0:TJ])
            nc.sync.dma_start(cos_sb[:], cos_d[:])
            nc.sync.dma_start(sin_sb[:], sin_d[:])
            nc.sync.dma_start(w_sb[:, 1, :, :], w_d[:, 1, :, :])
            nc.sync.dma_start(hst0[:, 16:24, :], hsT_d[:, 16:24, 0:TJ])
            nc.sync.dma_start(w_sb[:, 2, :, :], w_d[:, 2, :, :])
            nc.sync.dma_start(hst0[:, 24:32, :], hsT_d[:, 24:32, 0:TJ])
            nc.sync.dma_start(mask_sb[:], mask_d[:])
            nc.sync.dma_start(w_sb[:, 3, :, :], w_d[:, 3, :, :])
            nc.sync.dma_start(w_sb[:, 4, :, :], w_d[:, 4, :, :])
            nc.sync.dma_start(w_sb[:, 5, :, :], w_d[:, 5, :, :])
            nc.vector.memset(ones_sb[:], 1.0)
            make_identity(nc, ident_sb[:])

            # ---- o_proj unit machinery -------------------------------------
            # A unit computes out[t128, n*512:(n+1)*512] for one t block of
            # tile j from ot tiles + wo, staging into a [128, HID] bf16 tile
            # DMA'd per half-row. Units for tile j are emitted interleaved
            # into tile j+1's QKV/attention phases (j=NJ-1's at the end).
            pending = deque()
            ob_tiles = {}

            def emit_unit():
                if not pending:
                    return False
                jj, tl, n, ot_tiles = pending.popleft()
                key = (jj, tl)
                if key not in ob_tiles:
                    ob_tiles[key] = outp.tile(
                        [128, HID], BF16, tag="ob", name=f"ob{jj}_{tl}"
                    )
                ob = ob_tiles[key]
                ps = shp.tile([128, TJ], F32, tag="ps", name="ps_u")
                tloc = bass.ts(tl, 128)
                for c in range(NQH):
                    nc.tensor.matmul(
                        ps[:], ot_tiles[c][:, tloc], wo_sb[:, c, bass.ts(n, TJ)],
                        start=(c == 0), stop=(c == NQH - 1),
                    )
                if n % 2 == 0:
                    nc.scalar.copy(ob[:, bass.ts(n, TJ)], ps[:])
                else:
                    nc.vector.tensor_copy(ob[:, bass.ts(n, TJ)], ps[:])
                if n == HID // TJ - 1:
                    tg = 4 * jj + tl
                    nc.sync.dma_start(out_d[bass.ts(tg, 128), :], ob[:])
                    del ob_tiles[key]
                return True

            # cb emission order: k first (rope for k completes while q
            # matmuls run), then q0..q3, then v.  Host packs w in this order.
            hst_tiles = [hst0, None, None, None]

            for j in range(NJ):
                js = bass.ts(j, TJ)
                hst_j = hst_tiles[j]
                nblk = 4 * j + 4

                q_tiles = [
                    qp.tile([128, TJ], BF16, tag="q", name=f"q{j}_{h}")
                    for h in range(NQH)
                ]
                ot_tiles = [
                    otp.tile([128, TJ], BF16, tag="ot", name=f"ot{j}_{h}")
                    for h in range(NQH)
                ]

                # ---- QKV^T for this t tile (+ interleaved o_proj units) ----
                for cb in range(CB):
                    ps = shp.tile([128, TJ], F32, tag="ps", name="ps_qkv")
                    for h in range(HCH):
                        nc.tensor.matmul(
                            ps[:], w_sb[:, cb, h, :], hst_j[:, h, :],
                            start=(h == 0), stop=(h == HCH - 1),
                        )
                        if h in (11, 23):
                            emit_unit()
                    if cb < 5:
                        qk_t = qktp.tile([128, TJ], BF16, tag="qkt")
                        nc.scalar.copy(qk_t[:], ps[:])
                        # rope: x' = x*cos2 + swap(x)*sin2 (sin2 top negated)
                        swp = rtp.tile([128, TJ], BF16, tag="swp")
                        nc.vector.tensor_copy(swp[0:64, :], qk_t[64:128, :])
                        nc.vector.tensor_copy(swp[64:128, :], qk_t[0:64, :])
                        ta = rtp.tile([128, TJ], BF16, tag="ta")
                        nc.vector.tensor_tensor(
                            ta[:], qk_t[:], cos_sb[:, js], mybir.AluOpType.mult
                        )
                        tb = rtp.tile([128, TJ], BF16, tag="tb")
                        nc.vector.tensor_tensor(
                            tb[:], swp[:], sin_sb[:, js], mybir.AluOpType.mult
                        )
                        dst = kT_sb[:, js] if cb == 0 else q_tiles[cb - 1][:]
                        nc.vector.tensor_tensor(
                            dst, ta[:], tb[:], mybir.AluOpType.add
                        )
                    else:
                        nc.vector.tensor_copy(vT_sb[:, js], ps[:])

                # ---- V blocks for this tile ----
                for i in range(4 * j, 4 * j + 4):
                    pv = shp.tile([128, 128], BF16, tag="ps", name="ps_vt")
                    nc.tensor.transpose(pv[:], vT_sb[:, bass.ts(i, 128)], ident_sb[:])
                    nc.vector.tensor_copy(v_sb[i][:], pv[:])

                # prefetch hst for j+1; wo during j=0
                if j + 1 < NJ:
                    nxt = hstp.tile(
                        [128, HCH, TJ], BF16, tag="hst", name=f"hst{j + 1}"
                    )
                    njs = bass.ts(j + 1, TJ)
                    nc.sync.dma_start(nxt[:, 0:16, :], hsT_d[:, 0:16, njs])
                    nc.sync.dma_start(nxt[:, 16:32, :], hsT_d[:, 16:32, njs])
                    hst_tiles[j + 1] = nxt
                if j == 0:
                    nc.sync.dma_start(wo_sb[:], wo_d[:])

                # ---- attention (S two blocks ahead of PV; units fill) ----
                for h in range(NQH):
                    qT = q_tiles[h]
                    P = Pp.tile([128, SB, TJ], BF16, tag="P", name=f"P{j}_{h}")
                    ps_o = psop.tile([128, TJ], F32, tag="ps", name="ps_o")
                    acc = [None, None]
                    ps_s_tiles = {}

                    def do_s(i):
                        ps_s = pssp.tile([128, TJ], F32, tag="ps", name="ps_s")
                        nc.tensor.matmul(
                            ps_s[:], kT_sb[:, bass.ts(i, 128)], qT[:],
                            start=True, stop=True,
                        )
                        nc.scalar.activation(
                            P[:, i, :], ps_s[:],
                            mybir.ActivationFunctionType.Exp, scale=SCALE,
                        )
                        if i >= 4 * j:
                            nc.vector.tensor_tensor(
                                P[:, i, :], P[:, i, :], mask_sb[:, i - 4 * j, :],
                                mybir.AluOpType.mult,
                            )

                    def do_pv(i):
                        nc.tensor.matmul(
                            ps_o[:], v_sb[i][:], P[:, i, :],
                            start=(i == 0), stop=(i == nblk - 1),
                        )
                        a = i % 2
                        if acc[a] is None:
                            acc[a] = accp.tile(
                                [128, TJ], BF16, tag="acc", name=f"acc{a}"
                            )
                            nc.vector.tensor_copy(acc[a][:], P[:, i, :])
                        else:
                            nc.vector.tensor_tensor(
                                acc[a][:], acc[a][:], P[:, i, :],
                                mybir.AluOpType.add,
                            )

                    for i in range(nblk):
                        do_s(i)
                        if i >= 2:
                            do_pv(i - 2)
                            if i % 3 == 2:
                                emit_unit()
                    do_pv(nblk - 2)
                    do_pv(nblk - 1)

                    # denominator + normalize
                    pd = accp.tile([128, TJ], BF16, tag="acc", name="pd")
                    nc.vector.tensor_tensor(
                        pd[:], acc[0][:], acc[1][:], mybir.AluOpType.add
                    )
                    ps_den = shp.tile([128, TJ], F32, tag="ps", name="ps_den")
                    nc.tensor.matmul(
                        ps_den[:], ones_sb[:], pd[:], start=True, stop=True
                    )
                    # rc = 1/den via exp(-ln(den)) on ACT (ln and exp share an
                    # activation table; DVE InstReciprocal is 3.3us each and
                    # the custom-DVE fast ops don't compile on this walrus)
                    ld = rcpp.tile([128, TJ], F32, tag="rc", name="ld")
                    nc.scalar.activation(
                        ld[:], ps_den[:], mybir.ActivationFunctionType.Ln
                    )
                    rc = rcpp.tile([128, TJ], F32, tag="rc", name="rc")
                    nc.scalar.activation(
                        rc[:], ld[:], mybir.ActivationFunctionType.Exp,
                        scale=-1.0,
                    )
                    nc.vector.tensor_tensor(
                        ot_tiles[h][:], ps_o[:], rc[:], mybir.AluOpType.mult
                    )

                # drain leftover units from tile j-1, then queue tile j's
                while emit_unit():
                    pass
                for tl in range(4):
                    for n in range(HID // TJ):
                        pending.append((j, tl, n, ot_tiles))

            while emit_unit():
                pass

    _orig_to_json = nc.to_json_bytes

    def _patched_to_json():
        return _fix_bir_waits(_orig_to_json())

    nc.to_json_bytes = _patched_to_json
    return nc


_NC_CACHE = None


def _get_nc():
    global _NC_CACHE
    if _NC_CACHE is None:
        _NC_CACHE = build_nc()
    return _NC_CACHE


def _host_prep(positions, hidden_states, w_qkv, w_o):
    H, HKV = 32, 8
    pos = np.asarray(positions).astype(np.float32)
    inv_freq = 1.0 / (500000.0 ** (np.arange(0, D, 2, dtype=np.float32) / D))
    freqs = pos[:, None] * inv_freq[None, :]                  # [T, 64]
    cos = np.cos(freqs).T                                     # [64, T]
    sin = np.sin(freqs).T
    cos2 = np.ascontiguousarray(
        np.concatenate([cos, cos], 0)
    ).astype(bf16)                                            # [128, T]
    sin2 = np.ascontiguousarray(np.concatenate([-sin, sin], 0)).astype(bf16)

    # diagonal 0/1 masks: [p, r, f] = ((128r + p) <= f)
    p = np.arange(128)[:, None, None]
    r = np.arange(4)[None, :, None]
    f = np.arange(TJ)[None, None, :]
    masks = np.ascontiguousarray(
        ((128 * r + p) <= f).astype(np.float32)
    ).astype(bf16)                                            # [128, 4, 512]

    hs = np.asarray(hidden_states)
    # [p, o, t]
    hsT = np.ascontiguousarray(
        hs.T.reshape(HCH, 128, T).transpose(1, 0, 2)
    ).astype(bf16)
    w_qkv = np.asarray(w_qkv)
    w_o = np.asarray(w_o)

    in_maps = []
    for core in range(8):
        qc = slice(core * 4 * D, (core + 1) * 4 * D)
        kc = slice(H * D + core * D, H * D + (core + 1) * D)
        vc = slice((H + HKV) * D + core * D, (H + HKV) * D + (core + 1) * D)
        # cb order: k, q0..q3, v
        wshard = np.concatenate(
            [w_qkv[:, kc], w_qkv[:, qc], w_qkv[:, vc]], axis=1
        )                                                     # [4096, 768]
        # [p, cb, o, c]
        wshard = np.ascontiguousarray(
            wshard.reshape(HCH, 128, CB, 128).transpose(1, 2, 0, 3)
        ).astype(bf16)
        # [p, c, n]
        woshard = np.ascontiguousarray(
            w_o[core * 512:(core + 1) * 512, :]
            .reshape(NQH, 128, HID)
            .transpose(1, 0, 2)
        ).astype(bf16)
        in_maps.append(
            {
                "hsT": hsT,
                "wqkv": wshard,
                "wo": woshard,
                "cos2": cos2,
                "sin2": sin2,
                "masks": masks,
            }
        )
    return in_maps


def kernel(positions, hidden_states, w_qkv, w_o, _trace=False):
    nc = _get_nc()
    in_maps = _host_prep(positions, hidden_states, w_qkv, w_o)
    res = run_bass_kernel_spmd(nc, in_maps, list(range(8)), trace=_trace)
    out = np.zeros((T, HID), np.float32)
    for c in range(8):
        out += res.results[c]["out"].astype(np.float32)
    if _trace:
        kernel._last_result = res
    return out


# revision 12
# speedup vs baseline: 1.0358x; 1.0358x over previous
"""Llama3 attention layer (T=2048, 32 q heads / 8 kv heads, D=128, hidden 4096)
on 8 Trainium2 NeuronCores, tensor-parallel over heads.

Per-core shard: 4 q heads + 1 kv head (w_qkv columns), 512 w_o rows.
Each core computes a full [T, 4096] o_proj partial in bf16; the host sums
the 8 partials in f32 (the all-reduce of the row-parallel w_o).

Device algorithm (per core), matmuls in bf16 with fp32 PSUM accumulation:
  1. qkv^T = w_shard^T @ hs^T          -> [768, T]  (c on partitions)
     cb order: k first, then q0..q3, then v (k's rope off critical path)
  2. RoPE on q^T/k^T rows via duplicated cos/sin tables (DVE)
  3. V = transpose(v^T) via PE transposes
  4. S^T[s,t] = k^T.T @ q^T per head; exp on ACT into a wide P tile;
     0/1 mask on diagonal blocks (DVE)
  5. out^T[d,t] += V[s].T @ P^T[s,t]; denominator via DVE accumulation of
     P blocks + one ones-matmul per (head, tile); reciprocal_approx_fast;
     normalize reads PSUM directly
  6. o_proj emitted as 32 "units" per tile, interleaved into the NEXT
     tile's QKV/attention phases to keep the PE saturated
"""
import math
from collections import deque

import numpy as np
import ml_dtypes

import bass_rust
import concourse.bass as bass
import concourse.mybir as mybir
import concourse.tile as tile
from concourse.bass_utils import run_bass_kernel_spmd
from concourse.masks import make_identity
from concourse.vector_clock import ScopedClock

BF16 = mybir.dt.bfloat16
F32 = mybir.dt.float32
bf16 = ml_dtypes.bfloat16

T = 2048
HID = 4096
D = 128
NQH = 4           # q heads per core
CB = 6            # qkv col blocks of 128 (k, 4 q heads, v)
HCH = HID // 128  # 32 hidden chunks
TJ = 512          # t tile width
NJ = T // TJ      # 4 t tiles
SB = T // 128     # 16 s blocks
SCALE = 1.0 / math.sqrt(D)

_MAX_CTRL_WAITS = 1


def _install_drain_fix():
    """walrus in this image allows only 1 sem wait on CTRL (nop/drain)
    instructions; spread the Tile tail-drain's global-clock waits across
    preceding sync-engine NOPs."""
    if getattr(tile.TileContext, "_drain_fix_installed", False):
        return

    def _patched(self, tick_clock, wait_clock):
        nc = self.nc
        nops = [nc.sync.nop(nofuse=True, hint=f"drainw{i}") for i in range(32)]
        drain_inst = nc.sync.drain()
        wait_clock.add_sem_waits(
            drain_inst.ins, ScopedClock({None: tick_clock.global_clock})
        )
        si = drain_inst.ins.sync_info
        waits = list(si.on_wait) if si and si.on_wait else []
        if len(waits) > _MAX_CTRL_WAITS:
            chunks = [
                waits[i:i + _MAX_CTRL_WAITS]
                for i in range(0, len(waits), _MAX_CTRL_WAITS)
            ]
            drain_inst.ins.sync_info = bass_rust.SyncInfo(
                on_wait=chunks[-1], on_update=list(si.on_update or [])
            )
            for nop, chunk in zip(nops, chunks[:-1]):
                nop.ins.sync_info = bass_rust.SyncInfo(on_wait=chunk, on_update=[])
        nc.all_engine_barrier()
        assert self.sems is not None
        popped = nc._tile_sem_poison_stack.pop()
        assert popped is self._sem_poison
        nc.clear_and_free_semaphores(list(self.sems.allocated().values()))
        nc.all_engine_barrier()

    tile.TileContext._drain_and_barrier = _patched
    tile.TileContext._drain_fix_installed = True


def _fix_bir_waits(bir_json: bytes, max_waits: int = 1) -> bytes:
    """walrus in this image accepts very few sem waits per instruction.
    Split any instruction carrying more than `max_waits` waits by inserting
    same-engine NoOps ahead of it that carry the excess waits."""
    import json

    bir = json.loads(bir_json)
    for fn in bir["functions"]:
        for blk in fn["blocks"]:
            out = []
            for inst in blk["instructions"]:
                si = inst.get("sync_info")
                waits = (si or {}).get("on_wait") or []
                if len(waits) > max_waits:
                    chunks = [
                        waits[i:i + max_waits]
                        for i in range(0, len(waits), max_waits)
                    ]
                    for k, ch in enumerate(chunks[:-1]):
                        out.append(
                            {
                                "debug": inst.get("debug", 0),
                                "engine": inst["engine"],
                                "ins": [],
                                "name": f"{inst['name']}-w{k}",
                                "opcode": "NoOp",
                                "outs": [],
                                "sync_info": {"on_update": [], "on_wait": ch},
                            }
                        )
                    si["on_wait"] = chunks[-1]
                out.append(inst)
            blk["instructions"] = out
    return json.dumps(bir).encode()


def build_nc() -> bass.Bass:
    _install_drain_fix()
    nc = bass.Bass()

    hsT_d = nc.dram_tensor("hsT", [128, HCH, T], BF16, kind="ExternalInput")
    w_d = nc.dram_tensor("wqkv", [128, CB, HCH, 128], BF16, kind="ExternalInput")
    wo_d = nc.dram_tensor("wo", [128, NQH, HID], BF16, kind="ExternalInput")
    cos_d = nc.dram_tensor("cos2", [128, T], BF16, kind="ExternalInput")
    sin_d = nc.dram_tensor("sin2", [128, T], BF16, kind="ExternalInput")
    mask_d = nc.dram_tensor("masks", [128, 4, TJ], BF16, kind="ExternalInput")
    out_d = nc.dram_tensor("out", [T, HID], BF16, kind="ExternalOutput")

    with tile.TileContext(nc) as tc:
        with (
            tc.tile_pool(name="const", bufs=1) as constp,
            tc.tile_pool(name="acts", bufs=1) as actp,
            tc.tile_pool(name="hst", bufs=1) as hstp,
            tc.tile_pool(name="qp", bufs=8) as qp,
            tc.tile_pool(name="otp", bufs=8) as otp,
            tc.tile_pool(name="Pp", bufs=1) as Pp,
            tc.tile_pool(name="qkt", bufs=2) as qktp,
            tc.tile_pool(name="rtmp", bufs=3) as rtp,
            tc.tile_pool(name="accp", bufs=4) as accp,
            tc.tile_pool(name="rcp", bufs=4) as rcpp,
            tc.tile_pool(name="outp", bufs=2) as outp,
            tc.tile_pool(name="pss", bufs=2, space="PSUM") as pssp,
            tc.tile_pool(name="pso", bufs=2, space="PSUM") as psop,
            tc.tile_pool(name="shp", bufs=2, space="PSUM") as shp,
        ):
            cos_sb = constp.tile([128, T], BF16, tag="cos")
            sin_sb = constp.tile([128, T], BF16, tag="sin")
            mask_sb = constp.tile([128, 4, TJ], BF16, tag="mask")
            ones_sb = constp.tile([128, 128], BF16, tag="ones")
            ident_sb = constp.tile([128, 128], BF16, tag="ident")

            # persistent activations
            w_sb = actp.tile([128, CB, HCH, 128], BF16, tag="w")
            wo_sb = actp.tile([128, NQH, HID], BF16, tag="wo")
            kT_sb = actp.tile([128, T], BF16, tag="kT")
            vT_sb = actp.tile([128, T], BF16, tag="vT")
            v_sb = [
                actp.tile([128, 128], BF16, tag=f"v{i}", name=f"v{i}")
                for i in range(SB)
            ]

            # ---- initial DMA schedule (j=0): interleave w / hst / consts so
            # the first matmuls start as early as possible.
            hst0 = hstp.tile([128, HCH, TJ], BF16, tag="hst", name="hst0")
            for q in range(4):
                o8 = slice(8 * q, 8 * q + 8)
                nc.sync.dma_start(w_sb[:, 0, o8, :], w_d[:, 0, o8, :])
                nc.sync.dma_start(hst0[:, o8, :], hsT_d[:, o8, 0:TJ])
            nc.sync.dma_start(cos_sb[:], cos_d[:])
            nc.sync.dma_start(sin_sb[:], sin_d[:])
            nc.sync.dma_start(w_sb[:, 1, :, :], w_d[:, 1, :, :])
            nc.sync.dma_start(w_sb[:, 2, :, :], w_d[:, 2, :, :])
            nc.sync.dma_start(mask_sb[:], mask_d[:])
            nc.sync.dma_start(w_sb[:, 3, :, :], w_d[:, 3, :, :])
            nc.sync.dma_start(w_sb[:, 4, :, :], w_d[:, 4, :, :])
            nc.sync.dma_start(w_sb[:, 5, :, :], w_d[:, 5, :, :])
            nc.vector.memset(ones_sb[:], 1.0)
            make_identity(nc, ident_sb[:])

            # ---- o_proj unit machinery -------------------------------------
            # A unit computes out[t128, n*512:(n+1)*512] for one t block of
            # tile j from ot tiles + wo, staging into a [128, HID] bf16 tile
            # DMA'd per half-row. Units for tile j are emitted interleaved
            # into tile j+1's QKV/attention phases (j=NJ-1's at the end).
            pending = deque()
            ob_tiles = {}

            def emit_unit():
                if not pending:
                    return False
                jj, tl, n, ot_tiles = pending.popleft()
                key = (jj, tl)
                if key not in ob_tiles:
                    ob_tiles[key] = outp.tile(
                        [128, HID], BF16, tag="ob", name=f"ob{jj}_{tl}"
                    )
                ob = ob_tiles[key]
                ps = shp.tile([128, TJ], F32, tag="ps", name="ps_u")
                tloc = bass.ts(tl, 128)
                for c in range(NQH):
                    nc.tensor.matmul(
                        ps[:], ot_tiles[c][:, tloc], wo_sb[:, c, bass.ts(n, TJ)],
                        start=(c == 0), stop=(c == NQH - 1),
                    )
                if n % 2 == 0:
                    nc.scalar.copy(ob[:, bass.ts(n, TJ)], ps[:])
                else:
                    nc.vector.tensor_copy(ob[:, bass.ts(n, TJ)], ps[:])
                if n == 3 or n == HID // TJ - 1:
                    tg = 4 * jj + tl
                    half = bass.ts(n // 4, HID // 2)
                    nc.sync.dma_start(out_d[bass.ts(tg, 128), half], ob[:, half])
                    if n == HID // TJ - 1:
                        del ob_tiles[key]
                return True

            # cb emission order: k first (rope for k completes while q
            # matmuls run), then q0..q3, then v.  Host packs w in this order.
            hst_tiles = [hst0, None, None, None]

            for j in range(NJ):
                js = bass.ts(j, TJ)
                hst_j = hst_tiles[j]
                nblk = 4 * j + 4

                q_tiles = [
                    qp.tile([128, TJ], BF16, tag="q", name=f"q{j}_{h}")
                    for h in range(NQH)
                ]
                ot_tiles = [
                    otp.tile([128, TJ], BF16, tag="ot", name=f"ot{j}_{h}")
                    for h in range(NQH)
                ]

                # ---- QKV^T for this t tile (+ interleaved o_proj units) ----
                for cb in range(CB):
                    ps = shp.tile([128, TJ], F32, tag="ps", name="ps_qkv")
                    for h in range(HCH):
                        nc.tensor.matmul(
                            ps[:], w_sb[:, cb, h, :], hst_j[:, h, :],
                            start=(h == 0), stop=(h == HCH - 1),
                        )
                        if h in (11, 23):
                            emit_unit()
                    if cb < 5:
                        qk_t = qktp.tile([128, TJ], BF16, tag="qkt")
                        nc.scalar.copy(qk_t[:], ps[:])
                        # rope: x' = x*cos2 + swap(x)*sin2 (sin2 top negated)
                        swp = rtp.tile([128, TJ], BF16, tag="swp")
                        nc.vector.tensor_copy(swp[0:64, :], qk_t[64:128, :])
                        nc.vector.tensor_copy(swp[64:128, :], qk_t[0:64, :])
                        ta = rtp.tile([128, TJ], BF16, tag="ta")
                        nc.vector.tensor_tensor(
                            ta[:], qk_t[:], cos_sb[:, js], mybir.AluOpType.mult
                        )
                        tb = rtp.tile([128, TJ], BF16, tag="tb")
                        nc.vector.tensor_tensor(
                            tb[:], swp[:], sin_sb[:, js], mybir.AluOpType.mult
                        )
                        dst = kT_sb[:, js] if cb == 0 else q_tiles[cb - 1][:]
                        nc.vector.tensor_tensor(
                            dst, ta[:], tb[:], mybir.AluOpType.add
                        )
                    else:
                        nc.vector.tensor_copy(vT_sb[:, js], ps[:])

                # ---- V blocks for this tile ----
                for i in range(4 * j, 4 * j + 4):
                    pv = shp.tile([128, 128], BF16, tag="ps", name="ps_vt")
                    nc.tensor.transpose(pv[:], vT_sb[:, bass.ts(i, 128)], ident_sb[:])
                    nc.vector.tensor_copy(v_sb[i][:], pv[:])

                # prefetch hst for j+1; wo during j=0
                if j + 1 < NJ:
                    nxt = hstp.tile(
                        [128, HCH, TJ], BF16, tag="hst", name=f"hst{j + 1}"
                    )
                    njs = bass.ts(j + 1, TJ)
                    nc.sync.dma_start(nxt[:, 0:16, :], hsT_d[:, 0:16, njs])
                    nc.sync.dma_start(nxt[:, 16:32, :], hsT_d[:, 16:32, njs])
                    hst_tiles[j + 1] = nxt
                if j == 0:
                    nc.sync.dma_start(wo_sb[:], wo_d[:])

                # ---- attention: S-pairs one group ahead of PV; grouped exp
                # over [128, 1024] (2 PSUM banks) halves ACT per-tile cost;
                # o_proj units fill the PE between groups ----
                ngrp = nblk // 2
                for h in range(NQH):
                    qT = q_tiles[h]
                    P = Pp.tile([128, SB, TJ], BF16, tag="P", name=f"P{j}_{h}")
                    ps_o = psop.tile([128, TJ], F32, tag="ps", name="ps_o")
                    acc = [None, None]

                    def do_sgrp(g):
                        pg = pssp.tile([128, 2, TJ], F32, tag="ps", name="ps_s")
                        for u in range(2):
                            i = 2 * g + u
                            nc.tensor.matmul(
                                pg[:, u, :], kT_sb[:, bass.ts(i, 128)], qT[:],
                                start=True, stop=True,
                            )
                        nc.scalar.activation(
                            P[:, 2 * g:2 * g + 2, :], pg[:],
                            mybir.ActivationFunctionType.Exp, scale=SCALE,
                        )
                        if 2 * g >= 4 * j:
                            r = 2 * g - 4 * j
                            nc.vector.tensor_tensor(
                                P[:, 2 * g:2 * g + 2, :],
                                P[:, 2 * g:2 * g + 2, :],
                                mask_sb[:, r:r + 2, :],
                                mybir.AluOpType.mult,
                            )

                    def do_pv(i):
                        nc.tensor.matmul(
                            ps_o[:], v_sb[i][:], P[:, i, :],
                            start=(i == 0), stop=(i == nblk - 1),
                        )
                        a = i % 2
                        if acc[a] is None:
                            acc[a] = accp.tile(
                                [128, TJ], BF16, tag="acc", name=f"acc{a}"
                            )
                            nc.vector.tensor_copy(acc[a][:], P[:, i, :])
                        else:
                            nc.vector.tensor_tensor(
                                acc[a][:], acc[a][:], P[:, i, :],
                                mybir.AluOpType.add,
                            )

                    for g in range(ngrp):
                        do_sgrp(g)
                        if g >= 1:
                            do_pv(2 * g - 2)
                            do_pv(2 * g - 1)
                            if g % 2 == 1:
                                emit_unit()
                    do_pv(nblk - 2)
                    do_pv(nblk - 1)

                    # denominator + normalize
                    pd = accp.tile([128, TJ], BF16, tag="acc", name="pd")
                    nc.gpsimd.tensor_tensor(
                        pd[:], acc[0][:], acc[1][:], mybir.AluOpType.add
                    )
                    ps_den = psop.tile([128, TJ], F32, tag="ps", name="ps_den")
                    nc.tensor.matmul(
                        ps_den[:], ones_sb[:], pd[:], start=True, stop=True
                    )
                    # rc = 1/den via exp(-ln(den)) on ACT (ln and exp share an
                    # activation table; DVE InstReciprocal is 3.3us each and
                    # the custom-DVE fast ops don't compile on this walrus)
                    ld = rcpp.tile([128, TJ], F32, tag="rc", name="ld")
                    nc.scalar.activation(
                        ld[:], ps_den[:], mybir.ActivationFunctionType.Ln
                    )
                    rc = rcpp.tile([128, TJ], F32, tag="rc", name="rc")
                    nc.scalar.activation(
                        rc[:], ld[:], mybir.ActivationFunctionType.Exp,
                        scale=-1.0,
                    )
                    nc.vector.tensor_tensor(
                        ot_tiles[h][:], ps_o[:], rc[:], mybir.AluOpType.mult
                    )

                # drain leftover units from tile j-1, then queue tile j's
                while emit_unit():
                    pass
                for tl in range(4):
                    for n in range(HID // TJ):
                        pending.append((j, tl, n, ot_tiles))

            while emit_unit():
                pass

    _orig_to_json = nc.to_json_bytes

    def _patched_to_json():
        return _fix_bir_waits(_orig_to_json())

    nc.to_json_bytes = _patched_to_json
    return nc


_NC_CACHE = None


def _get_nc():
    global _NC_CACHE
    if _NC_CACHE is None:
        _NC_CACHE = build_nc()
    return _NC_CACHE


def _host_prep(positions, hidden_states, w_qkv, w_o):
    H, HKV = 32, 8
    pos = np.asarray(positions).astype(np.float32)
    inv_freq = 1.0 / (500000.0 ** (np.arange(0, D, 2, dtype=np.float32) / D))
    freqs = pos[:, None] * inv_freq[None, :]                  # [T, 64]
    cos = np.cos(freqs).T                                     # [64, T]
    sin = np.sin(freqs).T
    cos2 = np.ascontiguousarray(
        np.concatenate([cos, cos], 0)
    ).astype(bf16)                                            # [128, T]
    sin2 = np.ascontiguousarray(np.concatenate([-sin, sin], 0)).astype(bf16)

    # diagonal 0/1 masks: [p, r, f] = ((128r + p) <= f)
    p = np.arange(128)[:, None, None]
    r = np.arange(4)[None, :, None]
    f = np.arange(TJ)[None, None, :]
    masks = np.ascontiguousarray(
        ((128 * r + p) <= f).astype(np.float32)
    ).astype(bf16)                                            # [128, 4, 512]

    hs = np.asarray(hidden_states)
    # [p, o, t]
    hsT = np.ascontiguousarray(
        hs.T.reshape(HCH, 128, T).transpose(1, 0, 2)
    ).astype(bf16)
    w_qkv = np.asarray(w_qkv)
    w_o = np.asarray(w_o)

    in_maps = []
    for core in range(8):
        qc = slice(core * 4 * D, (core + 1) * 4 * D)
        kc = slice(H * D + core * D, H * D + (core + 1) * D)
        vc = slice((H + HKV) * D + core * D, (H + HKV) * D + (core + 1) * D)
        # cb order: k, q0..q3, v
        wshard = np.concatenate(
            [w_qkv[:, kc], w_qkv[:, qc], w_qkv[:, vc]], axis=1
        )                                                     # [4096, 768]
        # [p, cb, o, c]
        wshard = np.ascontiguousarray(
            wshard.reshape(HCH, 128, CB, 128).transpose(1, 2, 0, 3)
        ).astype(bf16)
        # [p, c, n]
        woshard = np.ascontiguousarray(
            w_o[core * 512:(core + 1) * 512, :]
            .reshape(NQH, 128, HID)
            .transpose(1, 0, 2)
        ).astype(bf16)
        in_maps.append(
            {
                "hsT": hsT,
                "wqkv": wshard,
                "wo": woshard,
                "cos2": cos2,
                "sin2": sin2,
                "masks": masks,
            }
        )
    return in_maps


def kernel(positions, hidden_states, w_qkv, w_o, _trace=False):
    nc = _get_nc()
    in_maps = _host_prep(positions, hidden_states, w_qkv, w_o)
    res = run_bass_kernel_spmd(nc, in_maps, list(range(8)), trace=_trace)
    out = np.zeros((T, HID), np.float32)
    for c in range(8):
        out += res.results[c]["out"].astype(np.float32)
    if _trace:
        kernel._last_result = res
    return out
